# revision 3
# baseline (speedup 1.0000x reference)
"""Conditional logistic regression forward on 8 Trainium2 NeuronCores.

out = y / segsum(y),  y = exp(x @ W + b),  segments sorted/contiguous.

Sharding: rows split into 8 contiguous equal chunks (one per core). Inside a
core, partition p owns rows [p*Fp, (p+1)*Fp) of the chunk (blocked layout).

Per-core device algorithm:
  z = x @ W          -- 64 accumulating fp32r matmuls, lhsT = W[d]*I (diagonal),
                        rhs = strided view x[:, :, d]; exact blocked layout out.
  y = exp(z + b)     -- ScalarE activation, PSUM -> SBUF.
  f = segmented prefix-sum of y (VectorE tensor_tensor_scan, mask resets)
  e = f * notm       -- segment totals at segment-end rows, 0 elsewhere
  A = reverse segmented scan of e -- broadcasts each segment's total to its rows
  carry fixups for segments straddling partition boundaries (edge-limited)
  out = y * reciprocal(A)

Segments straddling *core* boundaries (<= 7) are renormalized on the host
from the returned raw y. Host also fixes any partition-boundary segment
longer than the device edge window (EDGE), none expected in practice.
"""
import os
import sys
import types

import numpy as np

# ---- NTFF profile hook (axon image lacks antenv.axon_hooks; register our own)
def _ensure_profile_hook():
    if "antenv.axon_hooks" in sys.modules:
        return
    try:
        from trn_agent_boot.trn_boot import _ntff_profile_via_ctypes

        hook = _ntff_profile_via_ctypes("/opt/axon/libaxon_pjrt.so")
    except Exception:
        hook = None
    mod = types.ModuleType("antenv.axon_hooks")
    mod.get_axon_ntff_profile_hook = lambda: hook
    mod.set_axon_ntff_profile_hook = lambda h: None
    sys.modules["antenv.axon_hooks"] = mod


import concourse.bass as bass
import concourse.bacc as bacc
import concourse.tile as tile
from concourse import mybir

N = 4_194_304
D = 64
P = 128
NC = 8
R = N // NC          # 524288 rows per core
Fp = R // P          # 4096 rows per partition
Fs = 256             # rows per partition per subtile (matmul free dim)
EDGE = 512           # carry-apply window at partition edges (cols)

f32 = mybir.dt.float32
f32r = mybir.dt.float32r
u8 = mybir.dt.uint8
AL = mybir.AluOpType
AF = mybir.ActivationFunctionType

LAST_EXEC_NS = None


def _rev(ap_2d):
    """Negative-stride (reversed along last free dim) view of a 2D AP."""
    a = ap_2d.copy()
    steps = [list(sc) for sc in a.ap]
    assert len(steps) == 2, steps
    st, cnt = steps[1]
    return bass.AP(
        tensor=a.tensor, offset=a.offset + st * (cnt - 1),
        ap=[steps[0], [-st, cnt]],
    )


def _build(nc):
    nsub = Fp // Fs
    x_d = nc.dram_tensor("x", [R, D], f32r, kind="ExternalInput")
    wi_d = nc.dram_tensor("wi", [P, D, P], f32r, kind="ExternalInput")
    b_d = nc.dram_tensor("b", [P, 1], f32, kind="ExternalInput")
    m_d = nc.dram_tensor("m", [P, Fp + 4], u8, kind="ExternalInput")
    nm_d = nc.dram_tensor("nm", [P, Fp], u8, kind="ExternalInput")
    y_o = nc.dram_tensor("y_out", [P, Fp], f32, kind="ExternalOutput")
    o_o = nc.dram_tensor("o_out", [P, Fp], f32, kind="ExternalOutput")

    x_v = x_d.ap().rearrange("(p f) d -> p f d", p=P)

    with tile.TileContext(nc) as tc:
        with tc.tile_pool(name="keep", bufs=1) as sb:
            wi_sb = sb.tile([P, D, P], f32r)
            b_sb = sb.tile([P, 1], f32)
            m_sb = sb.tile([P, Fp + 4], u8)
            nm_sb = sb.tile([P, Fp], u8)
            y_sb = sb.tile([P, Fp], f32)
            vecs = sb.tile([P, 8], f32)

            # constants/metadata on the scalar HWDGE ring so they don't
            # serialize behind the x stream on the sync ring
            nc.scalar.dma_start(out=wi_sb, in_=wi_d.ap())
            nc.scalar.dma_start(out=b_sb, in_=b_d.ap())
            nc.scalar.dma_start(out=m_sb, in_=m_d.ap())
            nc.scalar.dma_start(out=nm_sb, in_=nm_d.ap())

            with (
                tc.tile_pool(name="xp", bufs=2) as xp,
                tc.tile_pool(name="psp", bufs=4, space="PSUM") as psp,
            ):
                for s in range(nsub):
                    x_t = xp.tile([P, Fs, D], f32r)
                    nc.sync.dma_start(out=x_t, in_=x_v[:, s * Fs : (s + 1) * Fs, :])
                    z_ps = psp.tile([P, Fs], f32)
                    for d in range(D):
                        nc.tensor.matmul(
                            z_ps, wi_sb[:, d, :], x_t[:, :, d],
                            start=(d == 0), stop=(d == D - 1),
                        )
                    nc.scalar.activation(
                        out=y_sb[:, s * Fs : (s + 1) * Fs], in_=z_ps, func=AF.Exp,
                        bias=b_sb[:, 0:1], scale=1.0,
                    )

            nc.sync.dma_start(out=y_o.ap(), in_=y_sb)

            with tc.tile_pool(name="tp", bufs=1) as tp:
                fe_sb = tp.tile([P, Fp], f32)
                a_sb = tp.tile([P, Fp], f32)
                ind_sb = tp.tile([P, EDGE], u8)

                m_f = m_sb[:, 0:Fp]
                m_b = m_sb[:, 1 : Fp + 1]

                # f = segmented prefix sum of y
                nc.vector.tensor_tensor_scan(
                    out=fe_sb, data0=m_f, data1=y_sb, initial=0.0,
                    op0=AL.mult, op1=AL.add,
                )
                # stash f[:, -1] and f32 gate m[:, 0]
                nc.vector.tensor_copy(vecs[:, 0:1], fe_sb[:, Fp - 1 : Fp])
                nc.vector.tensor_copy(vecs[:, 2:3], m_sb[:, 0:1])
                # e = f * notm (in place)
                nc.vector.tensor_mul(fe_sb, fe_sb, nm_sb)
                # A = reverse segmented broadcast of totals
                nc.vector.tensor_tensor_scan(
                    out=_rev(a_sb[:, :]), data0=_rev(m_b), data1=_rev(fe_sb[:, :]),
                    initial=0.0, op0=AL.mult, op1=AL.add,
                )
                # ind_first on the left edge window
                nc.vector.tensor_tensor_scan(
                    out=ind_sb, data0=m_sb[:, 0:EDGE], data1=m_sb[:, 0:EDGE],
                    initial=1.0, op0=AL.mult, op1=AL.mult,
                )
                # cin[p] = f_last[p-1] * m0[p]
                nc.vector.memset(vecs[:, 1:2], 0.0)
                nc.sync.dma_start(out=vecs[1:P, 1:2], in_=vecs[0 : P - 1, 0:1])
                nc.vector.tensor_mul(vecs[:, 1:2], vecs[:, 1:2], vecs[:, 2:3])
                nc.vector.scalar_tensor_tensor(
                    out=a_sb[:, 0:EDGE], in0=ind_sb, scalar=vecs[:, 1:2],
                    in1=a_sb[:, 0:EDGE], op0=AL.mult, op1=AL.add,
                )
                # ind_last on the right edge window
                nc.vector.tensor_tensor_scan(
                    out=_rev(ind_sb[:, :]),
                    data0=_rev(m_sb[:, Fp - EDGE + 1 : Fp + 1]),
                    data1=_rev(m_sb[:, Fp - EDGE + 1 : Fp + 1]),
                    initial=1.0, op0=AL.mult, op1=AL.mult,
                )
                # cout[p] = A[p+1, 0] * m0[p+1]
                nc.vector.tensor_mul(vecs[:, 3:4], a_sb[:, 0:1], vecs[:, 2:3])
                nc.vector.memset(vecs[:, 4:5], 0.0)
                nc.sync.dma_start(out=vecs[0 : P - 1, 4:5], in_=vecs[1:P, 3:4])
                nc.vector.scalar_tensor_tensor(
                    out=a_sb[:, Fp - EDGE : Fp], in0=ind_sb, scalar=vecs[:, 4:5],
                    in1=a_sb[:, Fp - EDGE : Fp], op0=AL.mult, op1=AL.add,
                )
                # out = y * (1/A); chunked so the out-DMA overlaps the DVE
                nch = 4
                w = Fp // nch
                for i in range(nch):
                    sl = slice(i * w, (i + 1) * w)
                    nc.vector.reciprocal_approx_fast(
                        out=fe_sb[:, sl], in_=a_sb[:, sl]
                    )
                    nc.vector.tensor_mul(a_sb[:, sl], y_sb[:, sl], fe_sb[:, sl])
                    nc.sync.dma_start(out=o_o.ap()[:, sl], in_=a_sb[:, sl])


_COMPILED_NC = None


def _get_nc():
    global _COMPILED_NC
    if _COMPILED_NC is None:
        nc = bacc.Bacc("TRN2", target_bir_lowering=False, debug=True)
        _build(nc)
        nc.compile()
        _COMPILED_NC = nc
    return _COMPILED_NC


def _host_prep_core(x_c, seg_c, wi, b_rep):
    M = np.zeros(R + 1, dtype=np.uint8)
    M[1:R] = seg_c[1:] == seg_c[:-1]
    base = (np.arange(P) * Fp)[:, None]
    m = np.zeros((P, Fp + 4), dtype=np.uint8)
    m[:, : Fp + 1] = M[base + np.arange(Fp + 1)[None, :]]
    m[0, 0] = 0
    nm = 1 - m[:, 1 : Fp + 1]
    return {"x": np.ascontiguousarray(x_c), "wi": wi, "b": b_rep, "m": m, "nm": nm}


def kernel(x, W, b, segment_ids):
    global LAST_EXEC_NS
    _ensure_profile_hook()
    from concourse.bass_utils import run_bass_kernel_spmd

    x = np.ascontiguousarray(np.asarray(x, dtype=np.float32))
    W = np.asarray(W, dtype=np.float32).reshape(D, 1)
    b = np.asarray(b, dtype=np.float32).reshape(1)
    seg = np.asarray(segment_ids)
    assert x.shape == (N, D) and seg.shape == (N,)

    wi = (np.eye(P, dtype=np.float32)[:, None, :] * W[:, 0][None, :, None])
    wi = np.ascontiguousarray(wi)  # [P, D, P]
    b_rep = np.full((P, 1), b[0], dtype=np.float32)

    in_maps = [
        _host_prep_core(x[c * R : (c + 1) * R], seg[c * R : (c + 1) * R], wi, b_rep)
        for c in range(NC)
    ]

    nc = _get_nc()
    trace = bool(int(os.environ.get("CLR_TRACE", "0")))
    trace_cores = None
    if trace:
        tc_env = os.environ.get("CLR_TRACE_CORES", "")
        if tc_env:
            trace_cores = [int(t) for t in tc_env.split(",")]
    res = run_bass_kernel_spmd(
        nc, in_maps, core_ids=list(range(NC)), trace=trace, trace_cores=trace_cores
    )
    LAST_EXEC_NS = res.exec_time_ns

    out = np.empty(N, dtype=np.float32)
    y = np.empty(N, dtype=np.float32)
    for c in range(NC):
        out[c * R : (c + 1) * R] = res.results[c]["o_out"].reshape(-1)
        y[c * R : (c + 1) * R] = res.results[c]["y_out"].reshape(-1)

    # host fixups: segments straddling core boundaries, plus any
    # partition-boundary segment longer than the device EDGE window.
    fix_rows = [c * R for c in range(1, NC)]
    fix_rows += [
        r for r in range(0, N, Fp) if r % R != 0
    ]
    fixed = set()
    for r in fix_rows:
        if seg[r] != seg[r - 1]:
            continue
        sid = seg[r]
        if sid in fixed:
            continue
        lo = int(np.searchsorted(seg, sid, "left"))
        hi = int(np.searchsorted(seg, sid, "right"))
        if r % R != 0 and (r - lo) <= EDGE and (hi - r) <= EDGE:
            # partition-boundary straddler inside the device edge windows
            continue
        fixed.add(sid)
        s = y[lo:hi].astype(np.float64).sum()
        out[lo:hi] = (y[lo:hi] / s).astype(np.float32)

    return out[:, None]


# revision 9
# speedup vs baseline: 1.0198x; 1.0198x over previous
"""Conditional logistic regression forward on 8 Trainium2 NeuronCores.

out = y / segsum(y),  y = exp(x @ W + b),  segments sorted/contiguous.

Sharding: rows split into 8 contiguous equal chunks (one per core). Inside a
core, partition p owns rows [p*Fp, (p+1)*Fp) of the chunk (blocked layout).

Per-core device algorithm:
  z = x @ W          -- 64 accumulating fp32r matmuls, lhsT = W[d]*I (diagonal),
                        rhs = strided view x[:, :, d]; exact blocked layout out.
  y = exp(z + b)     -- ScalarE activation, PSUM -> SBUF.
  f = segmented prefix-sum of y (VectorE tensor_tensor_scan, mask resets)
  e = f * notm       -- segment totals at segment-end rows, 0 elsewhere
  A = reverse segmented scan of e -- broadcasts each segment's total to its rows
  carry fixups for segments straddling partition boundaries (edge-limited)
  out = y * reciprocal(A)

Segments straddling *core* boundaries (<= 7) are renormalized on the host
from the returned raw y. Host also fixes any partition-boundary segment
longer than the device edge window (EDGE), none expected in practice.
"""
import os
import sys
import types

import numpy as np

# ---- NTFF profile hook (axon image lacks antenv.axon_hooks; register our own)
def _ensure_profile_hook():
    if "antenv.axon_hooks" in sys.modules:
        return
    try:
        from trn_agent_boot.trn_boot import _ntff_profile_via_ctypes

        hook = _ntff_profile_via_ctypes("/opt/axon/libaxon_pjrt.so")
    except Exception:
        hook = None
    mod = types.ModuleType("antenv.axon_hooks")
    mod.get_axon_ntff_profile_hook = lambda: hook
    mod.set_axon_ntff_profile_hook = lambda h: None
    sys.modules["antenv.axon_hooks"] = mod


import concourse.bass as bass
import concourse.bacc as bacc
import concourse.tile as tile
from concourse import mybir

N = int(os.environ.get("CLR_N", 4_194_304))
D = 64
P = 128
NC = 8
R = N // NC          # 524288 rows per core
Fp = R // P          # 4096 rows per partition
Fs = min(256, Fp)    # rows per partition per subtile (matmul free dim)
EDGE = min(512, Fp)  # carry-apply window at partition edges (cols)

f32 = mybir.dt.float32
f32r = mybir.dt.float32r
u8 = mybir.dt.uint8
AL = mybir.AluOpType
AF = mybir.ActivationFunctionType

LAST_EXEC_NS = None


def _rev(ap_2d):
    """Negative-stride (reversed along last free dim) view of a 2D AP."""
    a = ap_2d.copy()
    steps = [list(sc) for sc in a.ap]
    assert len(steps) == 2, steps
    st, cnt = steps[1]
    return bass.AP(
        tensor=a.tensor, offset=a.offset + st * (cnt - 1),
        ap=[steps[0], [-st, cnt]],
    )


def _build(nc):
    nsub = Fp // Fs
    x_d = nc.dram_tensor("x", [R, D], f32r, kind="ExternalInput")
    eye_d = nc.dram_tensor("eye", [P, P], f32, kind="ExternalInput")
    wr_d = nc.dram_tensor("wr", [P, D], f32, kind="ExternalInput")
    b_d = nc.dram_tensor("b", [P, 1], f32, kind="ExternalInput")
    m_d = nc.dram_tensor("m", [P, Fp + 4], u8, kind="ExternalInput")
    nm_d = nc.dram_tensor("nm", [P, Fp], u8, kind="ExternalInput")
    y_o = nc.dram_tensor("y_out", [P, Fp], f32, kind="ExternalOutput")
    o_o = nc.dram_tensor("o_out", [P, Fp], f32, kind="ExternalOutput")

    x_v = x_d.ap().rearrange("(p f) d -> p f d", p=P)

    with tile.TileContext(nc) as tc:
        with tc.tile_pool(name="keep", bufs=1) as sb:
            wi_sb = sb.tile([P, D, P], f32r)
            eye_sb = sb.tile([P, P], f32)
            wr_sb = sb.tile([P, D], f32)
            b_sb = sb.tile([P, 1], f32)
            m_sb = sb.tile([P, Fp + 4], u8)
            nm_sb = sb.tile([P, Fp], u8)
            y_sb = sb.tile([P, Fp], f32)
            fe_sb = sb.tile([P, Fp], f32)
            vecs = sb.tile([P, 8], f32)

            # constants/metadata on the scalar HWDGE ring so they don't
            # serialize behind the x stream on the sync ring
            nc.scalar.dma_start(out=eye_sb, in_=eye_d.ap())
            nc.scalar.dma_start(out=wr_sb, in_=wr_d.ap())
            nc.scalar.dma_start(out=b_sb, in_=b_d.ap())
            nc.scalar.dma_start(out=m_sb, in_=m_d.ap())
            nc.scalar.dma_start(out=nm_sb, in_=nm_d.ap())

            # build the 64 stationary diag matrices W[d]*I on-device
            for d in range(D):
                nc.vector.tensor_scalar_mul(
                    wi_sb[:, d, :], eye_sb, wr_sb[:, d : d + 1]
                )

            m_f = m_sb[:, 0:Fp]
            m_b = m_sb[:, 1 : Fp + 1]

            with (
                tc.tile_pool(name="xp", bufs=2) as xp,
                tc.tile_pool(name="psp", bufs=4, space="PSUM") as psp,
            ):
                for s in range(nsub):
                    sl = slice(s * Fs, (s + 1) * Fs)
                    x_t = xp.tile([P, Fs, D], f32r)
                    nc.sync.dma_start(out=x_t, in_=x_v[:, sl, :])
                    z_ps = psp.tile([P, Fs], f32)
                    for d in range(D):
                        nc.tensor.matmul(
                            z_ps, wi_sb[:, d, :], x_t[:, :, d],
                            start=(d == 0), stop=(d == D - 1),
                        )
                    nc.scalar.activation(
                        out=y_sb[:, sl], in_=z_ps, func=AF.Exp,
                        bias=b_sb[:, 0:1], scale=1.0,
                    )
                    # chained segmented prefix sum + segment-end extraction,
                    # overlapped under the DMA stream
                    nc.vector.tensor_tensor_scan(
                        out=fe_sb[:, sl], data0=m_sb[:, sl], data1=y_sb[:, sl],
                        initial=(0.0 if s == 0 else vecs[:, 5:6]),
                        op0=AL.mult, op1=AL.add,
                    )
                    nc.vector.tensor_copy(
                        vecs[:, 5:6], fe_sb[:, (s + 1) * Fs - 1 : (s + 1) * Fs]
                    )
                    # e = f * notm (in place) -- safe: carry already stashed
                    nc.vector.tensor_mul(fe_sb[:, sl], fe_sb[:, sl], nm_sb[:, sl])

            nc.sync.dma_start(out=y_o.ap(), in_=y_sb)

            with tc.tile_pool(name="tp", bufs=1) as tp:
                a_sb = tp.tile([P, Fp], f32)
                ind_sb = tp.tile([P, EDGE], u8)

                # f_last and f32 gate m[:, 0]
                nc.vector.tensor_copy(vecs[:, 0:1], vecs[:, 5:6])
                nc.vector.tensor_copy(vecs[:, 2:3], m_sb[:, 0:1])
                # A = reverse segmented broadcast of totals
                nc.vector.tensor_tensor_scan(
                    out=_rev(a_sb[:, :]), data0=_rev(m_b), data1=_rev(fe_sb[:, :]),
                    initial=0.0, op0=AL.mult, op1=AL.add,
                )
                # ind_first on the left edge window
                nc.vector.tensor_tensor_scan(
                    out=ind_sb, data0=m_sb[:, 0:EDGE], data1=m_sb[:, 0:EDGE],
                    initial=1.0, op0=AL.mult, op1=AL.mult,
                )
                # cin[p] = f_last[p-1] * m0[p]
                nc.vector.memset(vecs[:, 1:2], 0.0)
                nc.sync.dma_start(out=vecs[1:P, 1:2], in_=vecs[0 : P - 1, 0:1])
                nc.vector.tensor_mul(vecs[:, 1:2], vecs[:, 1:2], vecs[:, 2:3])
                nc.vector.scalar_tensor_tensor(
                    out=a_sb[:, 0:EDGE], in0=ind_sb, scalar=vecs[:, 1:2],
                    in1=a_sb[:, 0:EDGE], op0=AL.mult, op1=AL.add,
                )
                # ind_last on the right edge window
                nc.vector.tensor_tensor_scan(
                    out=_rev(ind_sb[:, :]),
                    data0=_rev(m_sb[:, Fp - EDGE + 1 : Fp + 1]),
                    data1=_rev(m_sb[:, Fp - EDGE + 1 : Fp + 1]),
                    initial=1.0, op0=AL.mult, op1=AL.mult,
                )
                # cout[p] = A[p+1, 0] * m0[p+1]
                nc.vector.tensor_mul(vecs[:, 3:4], a_sb[:, 0:1], vecs[:, 2:3])
                nc.vector.memset(vecs[:, 4:5], 0.0)
                nc.sync.dma_start(out=vecs[0 : P - 1, 4:5], in_=vecs[1:P, 3:4])
                nc.vector.scalar_tensor_tensor(
                    out=a_sb[:, Fp - EDGE : Fp], in0=ind_sb, scalar=vecs[:, 4:5],
                    in1=a_sb[:, Fp - EDGE : Fp], op0=AL.mult, op1=AL.add,
                )
                # out = y * (1/A); chunked so the out-DMA overlaps the DVE
                nch = 4
                w = Fp // nch
                for i in range(nch):
                    sl = slice(i * w, (i + 1) * w)
                    nc.vector.reciprocal_approx_fast(
                        out=fe_sb[:, sl], in_=a_sb[:, sl]
                    )
                    nc.vector.tensor_mul(a_sb[:, sl], y_sb[:, sl], fe_sb[:, sl])
                    nc.sync.dma_start(out=o_o.ap()[:, sl], in_=a_sb[:, sl])


_COMPILED_NC = None


def _get_nc():
    global _COMPILED_NC
    if _COMPILED_NC is None:
        nc = bacc.Bacc("TRN2", target_bir_lowering=False, debug=True)
        _build(nc)
        nc.compile()
        _COMPILED_NC = nc
    return _COMPILED_NC


def _host_prep_core(x_c, seg_c, shared):
    M = np.zeros(R + 1, dtype=np.uint8)
    M[1:R] = seg_c[1:] == seg_c[:-1]
    base = (np.arange(P) * Fp)[:, None]
    m = np.zeros((P, Fp + 4), dtype=np.uint8)
    m[:, : Fp + 1] = M[base + np.arange(Fp + 1)[None, :]]
    m[0, 0] = 0
    nm = 1 - m[:, 1 : Fp + 1]
    return {"x": np.ascontiguousarray(x_c), "m": m, "nm": nm, **shared}


def kernel(x, W, b, segment_ids):
    global LAST_EXEC_NS
    _ensure_profile_hook()
    from concourse.bass_utils import run_bass_kernel_spmd

    x = np.ascontiguousarray(np.asarray(x, dtype=np.float32))
    W = np.asarray(W, dtype=np.float32).reshape(D, 1)
    b = np.asarray(b, dtype=np.float32).reshape(1)
    seg = np.asarray(segment_ids)
    assert x.shape == (N, D) and seg.shape == (N,)

    shared = {
        "eye": np.eye(P, dtype=np.float32),
        "wr": np.ascontiguousarray(np.tile(W[:, 0], (P, 1))),
        "b": np.full((P, 1), b[0], dtype=np.float32),
    }

    in_maps = [
        _host_prep_core(x[c * R : (c + 1) * R], seg[c * R : (c + 1) * R], shared)
        for c in range(NC)
    ]

    nc = _get_nc()
    trace = bool(int(os.environ.get("CLR_TRACE", "0")))
    trace_cores = None
    if trace:
        tc_env = os.environ.get("CLR_TRACE_CORES", "")
        if tc_env:
            trace_cores = [int(t) for t in tc_env.split(",")]
    res = run_bass_kernel_spmd(
        nc, in_maps, core_ids=list(range(NC)), trace=trace, trace_cores=trace_cores
    )
    LAST_EXEC_NS = res.exec_time_ns

    out = np.empty(N, dtype=np.float32)
    y = np.empty(N, dtype=np.float32)
    for c in range(NC):
        out[c * R : (c + 1) * R] = res.results[c]["o_out"].reshape(-1)
        y[c * R : (c + 1) * R] = res.results[c]["y_out"].reshape(-1)

    # host fixups: segments straddling core boundaries, plus any
    # partition-boundary segment longer than the device EDGE window.
    fix_rows = [c * R for c in range(1, NC)]
    fix_rows += [
        r for r in range(0, N, Fp) if r % R != 0
    ]
    fixed = set()
    for r in fix_rows:
        if seg[r] != seg[r - 1]:
            continue
        sid = seg[r]
        if sid in fixed:
            continue
        lo = int(np.searchsorted(seg, sid, "left"))
        hi = int(np.searchsorted(seg, sid, "right"))
        if r % R != 0 and (r - lo) <= EDGE and (hi - r) <= EDGE:
            # partition-boundary straddler inside the device edge windows
            continue
        fixed.add(sid)
        s = y[lo:hi].astype(np.float64).sum()
        out[lo:hi] = (y[lo:hi] / s).astype(np.float32)

    return out[:, None]
